# revision 30
# baseline (speedup 1.0000x reference)
"""Trainium2 Bass kernel for nn_DigitConvolutionalModel (dense CNN -> MLP).

Pure data parallel over 8 NeuronCores (2048 samples each). The 3x3 conv is
linear, so the host folds it into the first FC layer (W1e = C @ w1.T), making
the whole network a 4-layer MLP computed in transposed orientation (features
on partitions, batch on the free dim) in fp16 (psum fp32, ~5e-4 rel err):

    outT = w4t.T @ relu(w3t.T @ relu(w2t.T @ relu(W1e.T @ xT + b1) + b2) + b3) + b4

Structure (evolved from the single-queue baseline via trace analysis):
  - Single need-ordered sync DMA queue (multi-queue splits the ~250 GB/s
    per-core HBM budget without raising it); wpack/bpack ride the idle
    gpsimd SWDGE queue.  DMA issues + PE warmup are emitted BEFORE the
    block so they start right after the framework preamble.
  - Tiles 0/1 run L1 with m0/m1 interleaved per chunk: two matmuls per
    arrived chunk halve the consumption rate, so PE never idles while
    chunks trickle in (PE idle gaps reset the HAM warm-up clock and can
    cost ~5us of half-clock matmuls).
  - ps3 double-buffered (8th PSUM bank) and tile-3's tail ops (D0/C1/B2)
    sandwiched into its m0 chunk waits, so only the last tile's
    r -> B -> h2 -> C -> h3 -> D -> out -> DMA chain is exposed at the end.
  - All kernel semaphores pinned to nums 208+ (the chunk of walrus's
    end-of-NEFF semaphore-zeroing epilogue cleared by the Sync engine,
    which is the engine still waiting on the final out-DMA).
"""

from contextlib import ExitStack

import ml_dtypes
import numpy as np

import concourse.bass as bass
import concourse.mybir as mybir

N_CORES = 8
B = 16384
BC = B // N_CORES
KC = 112
NKC = 7

TW = [512, 512, 512, 512]
TO = [0, 512, 1024, 1536]
NT = len(TW)

F32 = mybir.dt.float32
BF16 = mybir.dt.bfloat16
FP16 = mybir.dt.float16
RELU = mybir.ActivationFunctionType.Relu
ADD = mybir.AluOpType.add
MAX = mybir.AluOpType.max

N_WARM_MM = 8

# x chunk splits per tile (chunk axis), single sync queue in need order
X_SPLITS = [
    [(c, c + 1) for c in range(NKC)],      # t0 fine-grained
    [(0, 2), (2, 4), (4, 7)],
    [(0, 2), (2, 4), (4, 7)],
    [(0, 2), (2, 4), (4, 6), (6, 7)],
]
W1A = (0, 3)
W1B = (3, 7)

TAIL_ORDER = [
    ("B", 0), ("C", 0), ("B", 1), ("D", 0), ("C", 1), ("B", 2),
    ("D", 1), ("C", 2), ("B", 3), ("D", 2), ("C", 3), ("D", 3),
]
POS_PE = {op: i + 1 for i, op in enumerate(TAIL_ORDER)}

ACT_ORDER = [
    ("r", 0, 0), ("r", 0, 1), ("r", 1, 0), ("r", 1, 1), ("r", 2, 0),
    ("r", 2, 1), ("h3", 0), ("r", 3, 0), ("h3", 1), ("r", 3, 1),
    ("h3", 2), ("h3", 3),
]
POS_A = {op: i + 1 for i, op in enumerate(ACT_ORDER)}

DVE_ORDER = [
    ("h2", 0), ("h2", 1), ("out", 0), ("h2", 2), ("out", 1), ("h2", 3),
    ("out", 2), ("out", 3),
]
POS_V = {op: i + 1 for i, op in enumerate(DVE_ORDER)}

SEM_BASE = 208


def build_program(l1_dt=FP16, l234_dt=FP16):
    nc = bass.Bass()

    n_wp = 256 + 64 + 10

    xt_d = nc.declare_dram_parameter("xt", [KC, NKC, BC], l1_dt, isOutput=False)
    w1_d = nc.declare_dram_parameter("w1e", [KC, NKC * 256], l1_dt, isOutput=False)
    wp_d = nc.declare_dram_parameter("wpack", [128, n_wp], l234_dt, isOutput=False)
    bp_d = nc.declare_dram_parameter("bpack", [128, 5], F32, isOutput=False)
    out_d = nc.declare_dram_parameter("outT", [10, BC], FP16, isOutput=True)

    ctx = ExitStack()
    with ctx:
        xsb = ctx.enter_context(nc.sbuf_tensor([KC, NKC, BC], l1_dt))
        w1sb = ctx.enter_context(nc.sbuf_tensor([KC, NKC, 256], l1_dt))
        wpsb = ctx.enter_context(nc.sbuf_tensor([128, n_wp], l234_dt))
        bpsb = ctx.enter_context(nc.sbuf_tensor([128, 5], F32))
        h1sb = ctx.enter_context(nc.sbuf_tensor([128, 2, 2, 512], l234_dt))
        h2sb = ctx.enter_context(nc.sbuf_tensor([128, 2, 512], l234_dt))
        h3sb = ctx.enter_context(nc.sbuf_tensor([64, 2, 512], l234_dt))
        osb = ctx.enter_context(nc.sbuf_tensor([10, BC], FP16))
        warm = ctx.enter_context(nc.sbuf_tensor([1, 513], BF16))
        dump_a = ctx.enter_context(nc.sbuf_tensor([1, 16], BF16))
        dump_v = ctx.enter_context(nc.sbuf_tensor([1, 16], BF16))

        w2v = wpsb[:, 0:256].rearrange("p (c o) -> p c o", c=2)
        w3v = wpsb[:, 256:320]
        w4v = wpsb[0:64, 320:330]
        b1v = bpsb[:, 0:2]
        b2v = bpsb[:, 2:3]
        b3v = bpsb[0:64, 3:4]
        b4v = bpsb[0:10, 4:5]

        ps1 = ctx.enter_context(nc.psum_tensor([128, 2, 2, 512], F32))
        ps2 = ctx.enter_context(nc.psum_tensor([128, 512], F32))
        ps3 = ctx.enter_context(nc.psum_tensor([64, 2, 512], F32))
        ps4 = ctx.enter_context(nc.psum_tensor([10, 512], F32))

        nsem = iter(range(SEM_BASE, 256))

        def sem(name):
            return ctx.enter_context(nc.semaphore(name, num=next(nsem)))

        sg = sem("sg")
        swr = sem("swr")
        sw1a = sem("sw1a")
        sw1b = sem("sw1b")
        sx = [[sem(f"sx{t}_{g}") for g in range(len(X_SPLITS[t]))]
              for t in range(NT)]
        sm = sem("sm")
        s2 = sem("s2")
        sa = sem("sa")
        sv = sem("sv")
        sof = sem("sof")

        def xdma(eng, t, g):
            c0, c1 = X_SPLITS[t][g]
            o, w = TO[t], TW[t]
            eng.dma_start(
                out=xsb[:, c0:c1, o : o + w], in_=xt_d[:, c0:c1, o : o + w]
            ).then_inc(sx[t][g], 16)

        # Pre-block emission: DMA issues + PE warmup start right after the
        # framework preamble.
        a0, a1 = W1A
        nc.sync.dma_start(
            out=w1sb[:, a0:a1, :], in_=w1_d[:, a0 * 256 : a1 * 256]
        ).then_inc(sw1a, 16)
        xdma(nc.sync, 0, 0)
        xdma(nc.sync, 0, 1)
        b0, b1 = W1B
        nc.sync.dma_start(
            out=w1sb[:, b0:b1, :], in_=w1_d[:, b0 * 256 : b1 * 256]
        ).then_inc(sw1b, 16)
        for g in range(2, len(X_SPLITS[0])):
            xdma(nc.sync, 0, g)
        for t in range(1, NT):
            for g in range(len(X_SPLITS[t])):
                xdma(nc.sync, t, g)

        # warm tensor + small weights on gpsimd (its own SWDGE queue)
        nc.gpsimd.memset(warm[:], 0.125).then_inc(sg, 1)
        nc.gpsimd.dma_start(out=wpsb[:], in_=wp_d[:]).then_inc(swr, 16)
        nc.gpsimd.dma_start(out=bpsb[:], in_=bp_d[:]).then_inc(swr, 16)

        with nc.Block() as block:

            @block.sync
            def _(sy):
                for t in range(NT):
                    o, w = TO[t], TW[t]
                    sy.wait_ge(sv, POS_V[("out", t)])
                    sy.dma_start(
                        out=out_d[:, o : o + w], in_=osb[:, o : o + w]
                    ).then_inc(sof, 16)

            @block.scalar
            def _(se):
                se.wait_ge(sg, 1)
                se.activation(dump_a[:], warm[:, 0:16], RELU)  # relu table
                se.wait_ge(swr, 32)
                for op in ACT_ORDER:
                    if op[0] == "r":
                        _, t, m = op
                        st = t % 2
                        w = TW[t]
                        if t >= 2:
                            # h1sb[st] freed once B(t-2) consumed it
                            se.wait_ge(s2, POS_PE[("B", t - 2)])
                        se.wait_ge(sm, 2 * t + m + 1)
                        se.activation(
                            h1sb[:, st, m, 0:w], ps1[:, st, m, 0:w], RELU,
                            bias=b1v[:, m : m + 1],
                        ).then_inc(sa, 1)
                    else:
                        _, t = op
                        st = t % 2
                        w = TW[t]
                        se.wait_ge(s2, POS_PE[("C", t)])
                        se.activation(
                            h3sb[:, st, 0:w], ps3[:, st, 0:w], RELU,
                            bias=b3v[:],
                        ).then_inc(sa, 1)

            @block.vector
            def _(ve):
                ve.wait_ge(sg, 1)
                ve.tensor_scalar(dump_v[:], warm[:, 0:16], 0.0, 0.0, ADD, MAX)
                ve.wait_ge(swr, 32)
                for kind, t in DVE_ORDER:
                    st = t % 2
                    o, w = TO[t], TW[t]
                    if kind == "h2":
                        ve.wait_ge(s2, POS_PE[("B", t)])
                        ve.tensor_scalar(
                            h2sb[:, st, 0:w], ps2[:, 0:w], b2v[:], 0.0, ADD, MAX
                        ).then_inc(sv, 1)
                    else:
                        ve.wait_ge(s2, POS_PE[("D", t)])
                        ve.tensor_scalar(
                            osb[:, o : o + w], ps4[:, 0:w], b4v[:], None, ADD
                        ).then_inc(sv, 1)

            @block.tensor
            def _(te):
                te.wait_ge(sg, 1)

                def warm_mm(n):
                    for _i in range(n):
                        te.matmul(ps4[0:1, :], warm[:, 0:1], warm[:, 1:513],
                                  start=True, stop=True)

                warm_mm(N_WARM_MM)

                def a_waits(t, c):
                    for g, (cg0, _cg1) in enumerate(X_SPLITS[t]):
                        if cg0 == c:
                            te.wait_ge(sx[t][g], 16)
                    if t == 0:
                        if c == W1A[0]:
                            te.wait_ge(sw1a, 16)
                        if c == W1B[0]:
                            te.wait_ge(sw1b, 16)

                def a_mm(t, m, c):
                    st = t % 2
                    o, w = TO[t], TW[t]
                    mm = te.matmul(
                        ps1[:, st, m, 0:w],
                        w1sb[:, c, m * 128 : (m + 1) * 128],
                        xsb[:, c, o : o + w],
                        start=(c == 0),
                        stop=(c == NKC - 1),
                    )
                    if c == NKC - 1:
                        mm.then_inc(sm, 1)

                def emit_A_int(t, fillers=()):
                    # m0/m1 interleaved per chunk: jitter-tolerant, no
                    # PE idle gaps (gaps reset the HAM warm-up clock)
                    if t >= 2:
                        te.wait_ge(sa, POS_A[("r", t - 2, 1)])
                    for c in range(NKC):
                        a_waits(t, c)
                        a_mm(t, 0, c)
                        a_mm(t, 1, c)
                        if c in fillers:
                            warm_mm(1)

                def emit_A(t, m, cs=0, ce=NKC, ps_wait=True):
                    if ps_wait and t >= 2:
                        te.wait_ge(sa, POS_A[("r", t - 2, m)])
                    for c in range(cs, ce):
                        if m == 0:
                            a_waits(t, c)
                        a_mm(t, m, c)

                def emit_B(t):
                    st = t % 2
                    w = TW[t]
                    if t == 0:
                        te.wait_ge(swr, 32)
                    if t >= 1:
                        te.wait_ge(sv, POS_V[("h2", t - 1)])  # ps2 free
                    te.wait_ge(sa, POS_A[("r", t, 0)])
                    te.matmul(
                        ps2[:, 0:w], w2v[:, 0, :], h1sb[:, st, 0, 0:w],
                        start=True, stop=False,
                    )
                    te.wait_ge(sa, POS_A[("r", t, 1)])
                    te.matmul(
                        ps2[:, 0:w], w2v[:, 1, :], h1sb[:, st, 1, 0:w],
                        start=False, stop=True,
                    ).then_inc(s2, 1)

                def emit_C(t):
                    st = t % 2
                    w = TW[t]
                    if t >= 2:
                        te.wait_ge(sa, POS_A[("h3", t - 2)])  # ps3[st] free
                    te.wait_ge(sv, POS_V[("h2", t)])
                    te.matmul(
                        ps3[:, st, 0:w], w3v[:], h2sb[:, st, 0:w],
                        start=True, stop=True,
                    ).then_inc(s2, 1)

                def emit_D(t):
                    st = t % 2
                    w = TW[t]
                    if t >= 1:
                        te.wait_ge(sv, POS_V[("out", t - 1)])  # ps4 free
                    te.wait_ge(sa, POS_A[("h3", t)])
                    te.matmul(
                        ps4[:, 0:w], w4v[:], h3sb[:, st, 0:w],
                        start=True, stop=True,
                    ).then_inc(s2, 1)

                emit_A_int(0)
                emit_A_int(1)
                emit_B(0)
                emit_A_int(2)
                emit_C(0)
                emit_B(1)
                # t3: first chunk group of m0, tail ops ride any x stall
                te.wait_ge(sa, POS_A[("r", 1, 0)])
                emit_A(3, 0, 0, X_SPLITS[3][-1][0], ps_wait=False)
                emit_D(0)
                emit_C(1)
                emit_B(2)
                emit_A(3, 0, X_SPLITS[3][-1][0], NKC, ps_wait=False)
                emit_A(3, 1)
                emit_D(1)
                emit_C(2)
                emit_B(3)
                emit_D(2)
                emit_C(3)
                emit_D(3)

        # Post-block: only Sync still has work (the out-DMA HBM landing).
        nc.sync.wait_ge(sof, 16 * NT)

    return nc


def _np_dt(dt):
    if dt == BF16:
        return ml_dtypes.bfloat16
    if dt == FP16:
        return np.float16
    return np.float32


def prepare_inputs(x, conv_w, w1, b1, w2, b2, w3, b3, w4, b4,
                   l1_dt=FP16, l234_dt=FP16):
    w1v = np.ascontiguousarray(w1.T).reshape(26, 26, 256)
    w1e = np.zeros((28, 28, 256), dtype=np.float32)
    for di in range(3):
        for dj in range(3):
            w1e[di : di + 26, dj : dj + 26, :] += conv_w[di, dj] * w1v
    w1e = w1e.reshape(784, 256)
    w1t = np.ascontiguousarray(
        w1e.reshape(NKC, KC, 256).transpose(1, 0, 2)
    ).reshape(KC, NKC * 256).astype(_np_dt(l1_dt))

    w2t = np.ascontiguousarray(w2.T).reshape(2, 128, 128).transpose(1, 0, 2)
    wpack = np.zeros((128, 256 + 64 + 10), dtype=np.float32)
    wpack[:, 0:256] = w2t.reshape(128, 256)
    wpack[:, 256:320] = w3.T
    wpack[0:64, 320:330] = w4.T
    wpack = wpack.astype(_np_dt(l234_dt))

    bpack = np.zeros((128, 5), dtype=np.float32)
    bpack[:, 0:2] = b1.reshape(2, 128).T
    bpack[:, 2] = b2
    bpack[0:64, 3] = b3
    bpack[0:10, 4] = b4

    shared = {"w1e": w1t, "wpack": wpack, "bpack": bpack}
    in_maps = []
    for m in range(N_CORES):
        xc = x[m * BC : (m + 1) * BC]
        # [KC, NKC, BC]: xt[k, c, j] = xc[j, c*KC + k]
        xt = np.ascontiguousarray(
            xc.reshape(BC, NKC, KC).transpose(2, 1, 0)
        ).astype(_np_dt(l1_dt))
        in_maps.append({"xt": xt, **shared})
    return in_maps



_PROGRAM = None


def _get_program():
    global _PROGRAM
    if _PROGRAM is None:
        _PROGRAM = build_program()
    return _PROGRAM


def kernel(x, conv_w, w1, b1, w2, b2, w3, b3, w4, b4):
    from concourse import bass_utils

    args = [x, conv_w, w1, b1, w2, b2, w3, b3, w4, b4]
    x, conv_w, w1, b1, w2, b2, w3, b3, w4, b4 = [
        np.asarray(a, dtype=np.float32) for a in args
    ]
    nc = _get_program()
    in_maps = prepare_inputs(x, conv_w, w1, b1, w2, b2, w3, b3, w4, b4)
    res = bass_utils.run_bass_kernel_spmd(nc, in_maps, list(range(N_CORES)))
    out = np.concatenate(
        [np.ascontiguousarray(res.results[m]["outT"].T) for m in range(N_CORES)],
        axis=0,
    )
    return out.astype(np.float32)


# revision 32
# speedup vs baseline: 1.0410x; 1.0410x over previous
"""Trainium2 Bass kernel for nn_DigitConvolutionalModel (dense CNN -> MLP).

Pure data parallel over 8 NeuronCores (2048 samples each). The 3x3 conv is
linear, so the host folds it into the first FC layer (W1e = C @ w1.T), making
the whole network a 4-layer MLP computed in transposed orientation (features
on partitions, batch on the free dim) in fp16 (psum fp32, ~5e-4 rel err):

    outT = w4t.T @ relu(w3t.T @ relu(w2t.T @ relu(W1e.T @ xT + b1) + b2) + b3) + b4

Structure (evolved from the single-queue baseline via trace analysis):
  - Single need-ordered sync DMA queue (multi-queue splits the ~250 GB/s
    per-core HBM budget without raising it); wpack/bpack ride the idle
    gpsimd SWDGE queue.  DMA issues + PE warmup are emitted BEFORE the
    block so they start right after the framework preamble.
  - Tiles 0/1 run L1 with m0/m1 interleaved per chunk: two matmuls per
    arrived chunk halve the consumption rate, so PE never idles while
    chunks trickle in (PE idle gaps reset the HAM warm-up clock and can
    cost ~5us of half-clock matmuls).
  - ps3 double-buffered (8th PSUM bank) and tile-3's tail ops (D0/C1/B2)
    sandwiched into its m0 chunk waits, so only the last tile's
    r -> B -> h2 -> C -> h3 -> D -> out -> DMA chain is exposed at the end.
  - All kernel semaphores pinned to nums 208+ (the chunk of walrus's
    end-of-NEFF semaphore-zeroing epilogue cleared by the Sync engine,
    which is the engine still waiting on the final out-DMA).
"""

from contextlib import ExitStack

import ml_dtypes
import numpy as np

import concourse.bass as bass
import concourse.mybir as mybir

N_CORES = 8
B = 16384
BC = B // N_CORES
KC = 112
NKC = 7

TW = [512, 512, 512, 512]
TO = [0, 512, 1024, 1536]
NT = len(TW)

F32 = mybir.dt.float32
BF16 = mybir.dt.bfloat16
FP16 = mybir.dt.float16
RELU = mybir.ActivationFunctionType.Relu
ADD = mybir.AluOpType.add
MAX = mybir.AluOpType.max

N_WARM_MM = 8

# x chunk splits per tile (chunk axis), single sync queue in need order
X_SPLITS = [
    [(c, c + 1) for c in range(NKC)],      # t0 fine-grained
    [(0, 2), (2, 4), (4, 7)],
    [(0, 2), (2, 4), (4, 7)],
    [(0, 2), (2, 4), (4, 6), (6, 7)],
]
W1A = (0, 3)
W1B = (3, 7)

TAIL_ORDER = [
    ("B", 0), ("C", 0), ("B", 1), ("D", 0), ("C", 1), ("B", 2),
    ("D", 1), ("C", 2), ("B", 3), ("D", 2), ("C", 3), ("D", 3),
]
POS_PE = {op: i + 1 for i, op in enumerate(TAIL_ORDER)}

ACT_ORDER = [
    ("r", 0, 0), ("r", 0, 1), ("r", 1, 0), ("r", 1, 1), ("r", 2, 0),
    ("r", 2, 1), ("h3", 0), ("r", 3, 0), ("h3", 1), ("r", 3, 1),
    ("h3", 2), ("h3", 3),
]
POS_A = {op: i + 1 for i, op in enumerate(ACT_ORDER)}

DVE_ORDER = [
    ("h2", 0), ("h2", 1), ("out", 0), ("h2", 2), ("out", 1), ("h2", 3),
    ("out", 2), ("out", 3),
]
POS_V = {op: i + 1 for i, op in enumerate(DVE_ORDER)}

SEM_BASE = 208


def build_program(l1_dt=FP16, l234_dt=FP16):
    nc = bass.Bass()

    n_wp = 256 + 64 + 10

    xt_d = nc.declare_dram_parameter("xt", [KC, NKC, BC], l1_dt, isOutput=False)
    w1_d = nc.declare_dram_parameter("w1e", [KC, NKC * 256], l1_dt, isOutput=False)
    wp_d = nc.declare_dram_parameter("wpack", [128, n_wp], l234_dt, isOutput=False)
    bp_d = nc.declare_dram_parameter("bpack", [128, 5], F32, isOutput=False)
    out_d = nc.declare_dram_parameter("outT", [10, BC], FP16, isOutput=True)

    ctx = ExitStack()
    with ctx:
        xsb = ctx.enter_context(nc.sbuf_tensor([KC, NKC, BC], l1_dt))
        w1sb = ctx.enter_context(nc.sbuf_tensor([KC, NKC, 256], l1_dt))
        wpsb = ctx.enter_context(nc.sbuf_tensor([128, n_wp], l234_dt))
        bpsb = ctx.enter_context(nc.sbuf_tensor([128, 5], F32))
        h1sb = ctx.enter_context(nc.sbuf_tensor([128, 2, 2, 512], l234_dt))
        h2sb = ctx.enter_context(nc.sbuf_tensor([128, 2, 512], l234_dt))
        h3sb = ctx.enter_context(nc.sbuf_tensor([64, 2, 512], l234_dt))
        osb = ctx.enter_context(nc.sbuf_tensor([10, BC], FP16))
        warm = ctx.enter_context(nc.sbuf_tensor([1, 513], BF16))
        dump_a = ctx.enter_context(nc.sbuf_tensor([1, 16], BF16))
        dump_v = ctx.enter_context(nc.sbuf_tensor([1, 16], BF16))

        w2v = wpsb[:, 0:256].rearrange("p (c o) -> p c o", c=2)
        w3v = wpsb[:, 256:320]
        w4v = wpsb[0:64, 320:330]
        b1v = bpsb[:, 0:2]
        b2v = bpsb[:, 2:3]
        b3v = bpsb[0:64, 3:4]
        b4v = bpsb[0:10, 4:5]

        ps1 = ctx.enter_context(nc.psum_tensor([128, 2, 2, 512], F32))
        ps2 = ctx.enter_context(nc.psum_tensor([128, 512], F32))
        ps3 = ctx.enter_context(nc.psum_tensor([64, 2, 512], F32))
        ps4 = ctx.enter_context(nc.psum_tensor([10, 512], F32))

        nsem = iter(range(SEM_BASE, 256))

        def sem(name):
            return ctx.enter_context(nc.semaphore(name, num=next(nsem)))

        sg = sem("sg")
        swr = sem("swr")
        sw1a = sem("sw1a")
        sw1b = sem("sw1b")
        sx = [[sem(f"sx{t}_{g}") for g in range(len(X_SPLITS[t]))]
              for t in range(NT)]
        sm = sem("sm")
        s2 = sem("s2")
        sa = sem("sa")
        sv = sem("sv")
        sof = sem("sof")

        def xdma(eng, t, g):
            c0, c1 = X_SPLITS[t][g]
            o, w = TO[t], TW[t]
            eng.dma_start(
                out=xsb[:, c0:c1, o : o + w], in_=xt_d[:, c0:c1, o : o + w]
            ).then_inc(sx[t][g], 16)

        # Pre-block emission: DMA issues + PE warmup start right after the
        # framework preamble.
        a0, a1 = W1A
        nc.sync.dma_start(
            out=w1sb[:, a0:a1, :], in_=w1_d[:, a0 * 256 : a1 * 256]
        ).then_inc(sw1a, 16)
        xdma(nc.sync, 0, 0)
        xdma(nc.sync, 0, 1)
        b0, b1 = W1B
        nc.sync.dma_start(
            out=w1sb[:, b0:b1, :], in_=w1_d[:, b0 * 256 : b1 * 256]
        ).then_inc(sw1b, 16)
        for g in range(2, len(X_SPLITS[0])):
            xdma(nc.sync, 0, g)
        for t in range(1, NT):
            for g in range(len(X_SPLITS[t])):
                xdma(nc.sync, t, g)

        # warm tensor + small weights on gpsimd (its own SWDGE queue)
        nc.gpsimd.memset(warm[:], 0.125).then_inc(sg, 1)
        nc.gpsimd.dma_start(out=wpsb[:], in_=wp_d[:]).then_inc(swr, 16)
        nc.gpsimd.dma_start(out=bpsb[:], in_=bp_d[:]).then_inc(swr, 16)

        with nc.Block() as block:

            @block.sync
            def _(sy):
                for t in range(NT):
                    o, w = TO[t], TW[t]
                    sy.wait_ge(sv, POS_V[("out", t)])
                    sy.dma_start(
                        out=out_d[:, o : o + w], in_=osb[:, o : o + w]
                    ).then_inc(sof, 16)

            @block.scalar
            def _(se):
                se.wait_ge(sg, 1)
                se.activation(dump_a[:], warm[:, 0:16], RELU)  # relu table
                se.wait_ge(swr, 32)
                for op in ACT_ORDER:
                    if op[0] == "r":
                        _, t, m = op
                        st = t % 2
                        w = TW[t]
                        if t >= 2:
                            # h1sb[st] freed once B(t-2) consumed it
                            se.wait_ge(s2, POS_PE[("B", t - 2)])
                        se.wait_ge(sm, 2 * t + m + 1)
                        se.activation(
                            h1sb[:, st, m, 0:w], ps1[:, st, m, 0:w], RELU,
                            bias=b1v[:, m : m + 1],
                        ).then_inc(sa, 1)
                    else:
                        _, t = op
                        st = t % 2
                        w = TW[t]
                        se.wait_ge(s2, POS_PE[("C", t)])
                        se.activation(
                            h3sb[:, st, 0:w], ps3[:, st, 0:w], RELU,
                            bias=b3v[:],
                        ).then_inc(sa, 1)

            @block.vector
            def _(ve):
                ve.wait_ge(sg, 1)
                ve.tensor_scalar(dump_v[:], warm[:, 0:16], 0.0, 0.0, ADD, MAX)
                ve.wait_ge(swr, 32)
                for kind, t in DVE_ORDER:
                    st = t % 2
                    o, w = TO[t], TW[t]
                    if kind == "h2":
                        ve.wait_ge(s2, POS_PE[("B", t)])
                        ve.tensor_scalar(
                            h2sb[:, st, 0:w], ps2[:, 0:w], b2v[:], 0.0, ADD, MAX
                        ).then_inc(sv, 1)
                    else:
                        ve.wait_ge(s2, POS_PE[("D", t)])
                        ve.tensor_scalar(
                            osb[:, o : o + w], ps4[:, 0:w], b4v[:], None, ADD
                        ).then_inc(sv, 1)

            @block.tensor
            def _(te):
                te.wait_ge(sg, 1)

                def warm_mm(n):
                    for _i in range(n):
                        te.matmul(ps4[0:1, :], warm[:, 0:1], warm[:, 1:513],
                                  start=True, stop=True)

                warm_mm(N_WARM_MM)

                def a_waits(t, c):
                    for g, (cg0, _cg1) in enumerate(X_SPLITS[t]):
                        if cg0 == c:
                            te.wait_ge(sx[t][g], 16)
                    if t == 0:
                        if c == W1A[0]:
                            te.wait_ge(sw1a, 16)
                        if c == W1B[0]:
                            te.wait_ge(sw1b, 16)

                def a_mm(t, m, c):
                    st = t % 2
                    o, w = TO[t], TW[t]
                    mm = te.matmul(
                        ps1[:, st, m, 0:w],
                        w1sb[:, c, m * 128 : (m + 1) * 128],
                        xsb[:, c, o : o + w],
                        start=(c == 0),
                        stop=(c == NKC - 1),
                    )
                    if c == NKC - 1:
                        mm.then_inc(sm, 1)

                def emit_A_int(t, fillers=()):
                    # m0/m1 interleaved per chunk: jitter-tolerant, no
                    # PE idle gaps (gaps reset the HAM warm-up clock)
                    if t >= 2:
                        te.wait_ge(sa, POS_A[("r", t - 2, 1)])
                    for c in range(NKC):
                        a_waits(t, c)
                        a_mm(t, 0, c)
                        a_mm(t, 1, c)
                        if c in fillers:
                            warm_mm(1)

                def emit_A(t, m, cs=0, ce=NKC, ps_wait=True):
                    if ps_wait and t >= 2:
                        te.wait_ge(sa, POS_A[("r", t - 2, m)])
                    for c in range(cs, ce):
                        if m == 0:
                            a_waits(t, c)
                        a_mm(t, m, c)

                def emit_B(t):
                    st = t % 2
                    w = TW[t]
                    if t == 0:
                        te.wait_ge(swr, 32)
                    if t >= 1:
                        te.wait_ge(sv, POS_V[("h2", t - 1)])  # ps2 free
                    te.wait_ge(sa, POS_A[("r", t, 0)])
                    te.matmul(
                        ps2[:, 0:w], w2v[:, 0, :], h1sb[:, st, 0, 0:w],
                        start=True, stop=False,
                    )
                    te.wait_ge(sa, POS_A[("r", t, 1)])
                    te.matmul(
                        ps2[:, 0:w], w2v[:, 1, :], h1sb[:, st, 1, 0:w],
                        start=False, stop=True,
                    ).then_inc(s2, 1)

                def emit_C(t):
                    st = t % 2
                    w = TW[t]
                    if t >= 2:
                        te.wait_ge(sa, POS_A[("h3", t - 2)])  # ps3[st] free
                    te.wait_ge(sv, POS_V[("h2", t)])
                    te.matmul(
                        ps3[:, st, 0:w], w3v[:], h2sb[:, st, 0:w],
                        start=True, stop=True,
                    ).then_inc(s2, 1)

                def emit_D(t):
                    st = t % 2
                    w = TW[t]
                    if t >= 1:
                        te.wait_ge(sv, POS_V[("out", t - 1)])  # ps4 free
                    te.wait_ge(sa, POS_A[("h3", t)])
                    te.matmul(
                        ps4[:, 0:w], w4v[:], h3sb[:, st, 0:w],
                        start=True, stop=True,
                    ).then_inc(s2, 1)

                emit_A_int(0)
                emit_A_int(1)
                emit_B(0)
                emit_A_int(2)
                emit_C(0)
                emit_B(1)
                # t3: first chunk group of m0, tail ops ride any x stall
                te.wait_ge(sa, POS_A[("r", 1, 0)])
                emit_A(3, 0, 0, X_SPLITS[3][-1][0], ps_wait=False)
                emit_D(0)
                emit_C(1)
                emit_B(2)
                emit_A(3, 0, X_SPLITS[3][-1][0], NKC, ps_wait=False)
                emit_A(3, 1)
                emit_D(1)
                emit_C(2)
                emit_B(3)
                emit_D(2)
                emit_C(3)
                emit_D(3)

        # Post-block: only Sync still has work (the out-DMA HBM landing).
        nc.sync.wait_ge(sof, 16 * NT)

    return nc


def _np_dt(dt):
    if dt == BF16:
        return ml_dtypes.bfloat16
    if dt == FP16:
        return np.float16
    return np.float32


def prepare_inputs(x, conv_w, w1, b1, w2, b2, w3, b3, w4, b4,
                   l1_dt=FP16, l234_dt=FP16):
    w1v = np.ascontiguousarray(w1.T).reshape(26, 26, 256)
    w1e = np.zeros((28, 28, 256), dtype=np.float32)
    for di in range(3):
        for dj in range(3):
            w1e[di : di + 26, dj : dj + 26, :] += conv_w[di, dj] * w1v
    w1e = w1e.reshape(784, 256)
    w1t = np.ascontiguousarray(
        w1e.reshape(NKC, KC, 256).transpose(1, 0, 2)
    ).reshape(KC, NKC * 256).astype(_np_dt(l1_dt))

    w2t = np.ascontiguousarray(w2.T).reshape(2, 128, 128).transpose(1, 0, 2)
    wpack = np.zeros((128, 256 + 64 + 10), dtype=np.float32)
    wpack[:, 0:256] = w2t.reshape(128, 256)
    wpack[:, 256:320] = w3.T
    wpack[0:64, 320:330] = w4.T
    wpack = wpack.astype(_np_dt(l234_dt))

    bpack = np.zeros((128, 5), dtype=np.float32)
    bpack[:, 0:2] = b1.reshape(2, 128).T
    bpack[:, 2] = b2
    bpack[0:64, 3] = b3
    bpack[0:10, 4] = b4

    shared = {"w1e": w1t, "wpack": wpack, "bpack": bpack}
    in_maps = []
    for m in range(N_CORES):
        xc = x[m * BC : (m + 1) * BC]
        # [KC, NKC, BC]: xt[k, c, j] = xc[j, c*KC + k]
        xt = np.ascontiguousarray(
            xc.reshape(BC, NKC, KC).transpose(2, 1, 0)
        ).astype(_np_dt(l1_dt))
        in_maps.append({"xt": xt, **shared})
    return in_maps



_PROGRAM = None


def _get_program():
    global _PROGRAM
    if _PROGRAM is None:
        _PROGRAM = build_program()
    return _PROGRAM


def kernel(x, conv_w, w1, b1, w2, b2, w3, b3, w4, b4):
    from concourse import bass_utils

    args = [x, conv_w, w1, b1, w2, b2, w3, b3, w4, b4]
    x, conv_w, w1, b1, w2, b2, w3, b3, w4, b4 = [
        np.asarray(a, dtype=np.float32) for a in args
    ]
    nc = _get_program()
    in_maps = prepare_inputs(x, conv_w, w1, b1, w2, b2, w3, b3, w4, b4)
    res = bass_utils.run_bass_kernel_spmd(nc, in_maps, list(range(N_CORES)))
    out = np.concatenate(
        [np.ascontiguousarray(res.results[m]["outT"].T) for m in range(N_CORES)],
        axis=0,
    )
    return out.astype(np.float32)
